# revision 9
# baseline (speedup 1.0000x reference)
"""Event-to-image scatter kernel for Trainium2 (Bass/Tile), 8-core SPMD.

Problem: x [16, 500000, 4] f32 events (t, x, y, p) -> [16, 720, 1280, 3] f32.
Per batch: ch0 = 255 except 0 where last event at pixel has p==1; ch1 = 255
except 0 where last p==0; ch2 = ch0 + ch1. Last-write-wins on duplicate
pixels (event order).

Sharding: pure data parallel - batch dim across 8 cores (2 batches/core).

The kernel is bound by the Pool engine's SWDGE descriptor generation for the
row-indirect scatter DMAs: each instruction carries at most 128 indexed
4-byte writes (one index per partition - a hardware limit of the indirect
DMA path) and costs ~1.42us on the Pool sequencer (994ns fixed ucode
overhead + per-descriptor + dispatch), measured back-to-back with no
dependencies. ~7810 instructions/core => ~11.2ms floor for this approach.

Device algorithm per batch:
  1. Decode (DVE): per event, pixel offset q = floor(y)*1280 + floor(x)
     (exact floor via rne-int + is_gt correction) and polarity p, packed as
     one f32 plane packed = q*4 + p (fraction < 0.5 so later rne is exact).
     Events are carved into 126 runs of 3968 (31 full 128-blocks each) so
     every scatter instruction carries a full 128 events; the 32 leftover
     events go to a tail instruction.
  2. Transpose the packed plane in 128-blocks (PE transpose; values < 2^24
     so f32 is exact), interleaved with the decode chunks. After
     transposing, column 126*k + r holds events [r*3968 + 128k, +128).
  3. Scatter: one row-indirect SWDGE DMA per 128 consecutive events, issued
     in ascending global order; Tile's WAW deps on the image serialize each
     chain. Each batch splits into 2 half-chains (63 runs each = contiguous
     global event ranges) on 2 images; the dense pass merges half-1 over
     half-0, preserving exact last-write-wins. Batch 0's scatters are all
     issued before batch 1's, so batch 0's dense pass overlaps batch 1's
     scatter stream.
  4. Dense pass: reload the 0/1/2 images, merge halves, build the three
     channels with DVE ops, write interleaved [720, 1280, 3].
"""
import numpy as np

W, H = 1280, 720
B, N = 16, 500000
NPIX = H * W
P = 128
NRUN = 126          # event runs per batch
S = 3968            # events per run (126*3968 = 499968), 31 full 128-blocks
TAIL = N - NRUN * S  # 32
NBLK = 31
NCORES = 8
BPC = B // NCORES   # batches per core
NQ = 3              # chains per batch
HRUN = NRUN // NQ   # runs per chain (42)

_compiled = None


def _build():
    from concourse import bacc, bass, mybir, tile
    from concourse.masks import make_identity

    nc = bacc.Bacc("TRN2", target_bir_lowering=False, debug=False,
                   num_swdge_queues=2)
    x_d = nc.dram_tensor("x", [BPC, N, 4], mybir.dt.float32, kind="ExternalInput")
    out_d = nc.dram_tensor("out", [BPC, H, W, 3], mybir.dt.float32,
                           kind="ExternalOutput")
    imgs = [nc.dram_tensor(f"img{i}", [NPIX, 1], mybir.dt.int32)
            for i in range(BPC * NQ)]

    f32, i32 = mybir.dt.float32, mybir.dt.int32
    CH = 496        # decode chunk: 8 chunks of 496 = 3968 columns
    NCH = S // CH

    with tile.TileContext(nc) as tc:
        with tc.tile_pool(name="sbuf", bufs=2) as pool, \
             tc.tile_pool(name="persist", bufs=1) as pp, \
             tc.tile_pool(name="psum", bufs=2, space="PSUM") as psp:
            ident = pp.tile([P, P], f32)
            make_identity(nc, ident[:])
            ztile = pp.tile([P, 1200], i32)
            nc.vector.memset(ztile[:], 0)
            # image zeroing on the (otherwise idle) Activation engine's DMA
            # queue so the sync queue starts decode input loads immediately
            inits = [[] for _ in range(BPC * NQ)]
            for i in range(BPC * NQ):
                for k in range(6):
                    hz = nc.scalar.dma_start(
                        out=imgs[i].ap()[k * 153600:(k + 1) * 153600, :]
                        .rearrange("(p f) o -> p (f o)", p=P),
                        in_=ztile[:],
                    )
                    inits[i].append(hz.ins)

            # transposed planes: column 126*k + r holds events
            # [r*S + 128k, +128) of the batch
            offsT = [pp.tile([P, NBLK * NRUN], i32, name=f"offsT{b}",
                             tag=f"offsT{b}") for b in range(BPC)]
            valsT = [pp.tile([P, NBLK * NRUN], i32, name=f"valsT{b}",
                             tag=f"valsT{b}") for b in range(BPC)]
            tails = []

            def transpose_block(b, packed, k):
                c0 = k * NRUN
                pt = psp.tile([P, NRUN], f32, tag="pt", space="PSUM")
                nc.tensor.transpose(out=pt[:, :],
                                    in_=packed[:NRUN, k * P:(k + 1) * P],
                                    identity=ident[:NRUN, :NRUN])
                tp = pool.tile([P, NRUN], f32, tag="tp")
                of = pool.tile([P, NRUN], f32, tag="of")
                nc.vector.tensor_copy(out=tp[:], in_=pt[:, :])
                # off = rne(packed/4) since frac is 0 or .25
                nc.vector.tensor_scalar(out=of[:], in0=tp[:],
                                        scalar1=0.25, scalar2=None,
                                        op0=mybir.AluOpType.mult)
                nc.vector.tensor_copy(
                    out=offsT[b][:, c0:c0 + NRUN], in_=of[:])
                ofr = pool.tile([P, NRUN], f32, tag="ofr")
                nc.vector.tensor_copy(
                    out=ofr[:], in_=offsT[b][:, c0:c0 + NRUN])
                # val = packed - 4*off + 1  (= p + 1)
                nc.vector.tensor_scalar(out=ofr[:], in0=ofr[:],
                                        scalar1=-4.0, scalar2=1.0,
                                        op0=mybir.AluOpType.mult,
                                        op1=mybir.AluOpType.add)
                nc.vector.tensor_add(out=ofr[:], in0=ofr[:], in1=tp[:])
                nc.vector.tensor_copy(
                    out=valsT[b][:, c0:c0 + NRUN], in_=ofr[:])

            def decode_batch(b):
                packed = pool.tile([P, S], f32, tag="packed", bufs=1)
                kdone = 0
                for ch in range(NCH):
                    e0 = ch * CH
                    raw = pool.tile([P, CH * 4], f32, tag="raw")
                    src = x_d.ap()[b, :NRUN * S, :].rearrange(
                        "(p s) f -> p (s f)", p=NRUN)
                    nc.sync.dma_start(out=raw[:NRUN, :],
                                      in_=src[:, e0 * 4:(e0 + CH) * 4])
                    xs = raw[:NRUN, 1::4]
                    ys = raw[:NRUN, 2::4]
                    ps = raw[:NRUN, 3::4]
                    fx = pool.tile([P, CH], f32, tag="fx")
                    fy = pool.tile([P, CH], f32, tag="fy")
                    ti = pool.tile([P, CH], i32, tag="ti")
                    tg = pool.tile([P, CH], f32, tag="tg")
                    # exact floor(xs)
                    nc.vector.tensor_copy(out=ti[:NRUN, :], in_=xs)
                    nc.vector.tensor_copy(out=fx[:NRUN, :], in_=ti[:NRUN, :])
                    nc.vector.tensor_tensor(out=tg[:NRUN, :], in0=fx[:NRUN, :],
                                            in1=xs, op=mybir.AluOpType.is_gt)
                    nc.vector.tensor_sub(out=fx[:NRUN, :], in0=fx[:NRUN, :],
                                         in1=tg[:NRUN, :])
                    # exact floor(ys)
                    nc.vector.tensor_copy(out=ti[:NRUN, :], in_=ys)
                    nc.vector.tensor_copy(out=fy[:NRUN, :], in_=ti[:NRUN, :])
                    nc.vector.tensor_tensor(out=tg[:NRUN, :], in0=fy[:NRUN, :],
                                            in1=ys, op=mybir.AluOpType.is_gt)
                    nc.vector.tensor_sub(out=fy[:NRUN, :], in0=fy[:NRUN, :],
                                         in1=tg[:NRUN, :])
                    # packed = (fy*1280 + fx)*4 + p  (exact in f32)
                    nc.vector.tensor_scalar(out=fy[:NRUN, :], in0=fy[:NRUN, :],
                                            scalar1=float(W), scalar2=None,
                                            op0=mybir.AluOpType.mult)
                    nc.vector.tensor_add(out=fy[:NRUN, :], in0=fy[:NRUN, :],
                                         in1=fx[:NRUN, :])
                    nc.vector.tensor_scalar(out=fy[:NRUN, :], in0=fy[:NRUN, :],
                                            scalar1=4.0, scalar2=None,
                                            op0=mybir.AluOpType.mult)
                    nc.vector.tensor_add(out=packed[:NRUN, e0:e0 + CH],
                                         in0=fy[:NRUN, :], in1=ps)
                    # rows 126..127 of packed are never read (transpose uses
                    # [:NRUN]); leave them.
                    # transpose every block fully covered by decoded columns
                    kready = ((ch + 1) * CH) // P if ch < NCH - 1 else NBLK
                    while kdone < kready:
                        transpose_block(b, packed, kdone)
                        kdone += 1

                # tail: events 499968..500000 (32) -> partitions 0..31
                toff = pp.tile([32, 1], i32, name=f"toff{b}", tag=f"toff{b}")
                tval = pp.tile([32, 1], i32, name=f"tval{b}", tag=f"tval{b}")
                traw = pool.tile([32, 4], f32, tag="traw")
                nc.sync.dma_start(out=traw[:],
                                  in_=x_d.ap()[b, NRUN * S:NRUN * S + TAIL, :])
                txs, tys, tps = traw[:, 1:2], traw[:, 2:3], traw[:, 3:4]
                tfx = pool.tile([32, 1], f32, tag="tfx")
                tfy = pool.tile([32, 1], f32, tag="tfy")
                tti = pool.tile([32, 1], i32, tag="tti")
                ttg = pool.tile([32, 1], f32, tag="ttg")
                nc.vector.tensor_copy(out=tti[:], in_=txs)
                nc.vector.tensor_copy(out=tfx[:], in_=tti[:])
                nc.vector.tensor_tensor(out=ttg[:], in0=tfx[:], in1=txs,
                                        op=mybir.AluOpType.is_gt)
                nc.vector.tensor_sub(out=tfx[:], in0=tfx[:], in1=ttg[:])
                nc.vector.tensor_copy(out=tti[:], in_=tys)
                nc.vector.tensor_copy(out=tfy[:], in_=tti[:])
                nc.vector.tensor_tensor(out=ttg[:], in0=tfy[:], in1=tys,
                                        op=mybir.AluOpType.is_gt)
                nc.vector.tensor_sub(out=tfy[:], in0=tfy[:], in1=ttg[:])
                nc.vector.tensor_scalar(out=tfy[:], in0=tfy[:],
                                        scalar1=float(W), scalar2=None,
                                        op0=mybir.AluOpType.mult)
                nc.vector.tensor_add(out=tfy[:], in0=tfy[:], in1=tfx[:])
                nc.vector.tensor_copy(out=toff[:], in_=tfy[:])
                nc.vector.tensor_scalar(out=ttg[:], in0=tps, scalar1=1.0,
                                        scalar2=None, op0=mybir.AluOpType.add)
                nc.vector.tensor_copy(out=tval[:], in_=ttg[:])
                tails.append((toff, tval))

            def scatter_batch(b):
                """NQ interleaved half-chains, global event order per half."""
                last = [None] * NQ
                first_done = set()
                for r in range(HRUN):
                    for k in range(NBLK):
                        for hf in range(NQ):
                            col = k * NRUN + HRUN * hf + r
                            im = b * NQ + hf
                            h = nc.gpsimd.indirect_dma_start(
                                out=imgs[im].ap(),
                                out_offset=bass.IndirectOffsetOnAxis(
                                    ap=offsT[b][:, col:col + 1], axis=0),
                                in_=valsT[b][:, col:col + 1],
                                in_offset=None,
                            )
                            if b:
                                h.ins.queue = f"qPoolDynamic{b}"
                            if im not in first_done:
                                for ins0 in inits[im]:
                                    tile.add_dep_helper(
                                        h.ins, ins0, reason="init before scatter")
                                first_done.add(im)
                            last[hf] = h.ins
                toff, tval = tails[b]
                h = nc.gpsimd.indirect_dma_start(
                    out=imgs[b * NQ + NQ - 1].ap(),
                    out_offset=bass.IndirectOffsetOnAxis(ap=toff[:, :1], axis=0),
                    in_=tval[:, :1],
                    in_offset=None,
                )
                if b:
                    h.ins.queue = f"qPoolDynamic{b}"
                last[NQ - 1] = h.ins
                return last

            def dense_batch(b, last):
                """merged img (0/1/2) -> [H, W, 3]"""
                for t in range(6):
                    rows = 128 if t < 5 else 80
                    r0 = t * 128
                    a_i = pool.tile([P, W], i32, tag="da")
                    b_i = pool.tile([P, W], i32, tag="db")
                    m_i = pool.tile([P, W], i32, tag="dm")
                    a_f = pool.tile([P, W], f32, tag="df")
                    m = pool.tile([P, W], f32, tag="dg")
                    ot = pool.tile([P, 3 * W], f32, tag="ot")
                    ld = nc.sync.dma_start(
                        out=a_i[:rows, :],
                        in_=imgs[b * NQ].ap()[r0 * W:(r0 + rows) * W, :]
                        .rearrange("(p f) o -> p (f o)", p=rows),
                    )
                    tile.add_dep_helper(ld.ins, last[0],
                                        reason="scatter before read")
                    for hf in range(1, NQ):
                        ld2 = nc.sync.dma_start(
                            out=b_i[:rows, :],
                            in_=imgs[b * NQ + hf].ap()[r0 * W:(r0 + rows) * W, :]
                            .rearrange("(p f) o -> p (f o)", p=rows),
                        )
                        tile.add_dep_helper(ld2.ins, last[hf],
                                            reason="scatter before read")
                        nc.vector.tensor_scalar(out=m_i[:rows, :],
                                                in0=b_i[:rows, :],
                                                scalar1=0, scalar2=None,
                                                op0=mybir.AluOpType.is_gt)
                        nc.vector.select(out=a_i[:rows, :], mask=m_i[:rows, :],
                                         on_true=b_i[:rows, :],
                                         on_false=a_i[:rows, :])
                    nc.vector.tensor_copy(out=a_f[:rows, :], in_=a_i[:rows, :])
                    nc.vector.tensor_scalar(out=m[:rows, :], in0=a_f[:rows, :],
                                            scalar1=2.0, scalar2=-255.0,
                                            op0=mybir.AluOpType.is_equal,
                                            op1=mybir.AluOpType.mult)
                    nc.vector.tensor_scalar(out=ot[:rows, 0::3], in0=m[:rows, :],
                                            scalar1=255.0, scalar2=None,
                                            op0=mybir.AluOpType.add)
                    nc.vector.tensor_scalar(out=m[:rows, :], in0=a_f[:rows, :],
                                            scalar1=1.0, scalar2=-255.0,
                                            op0=mybir.AluOpType.is_equal,
                                            op1=mybir.AluOpType.mult)
                    nc.vector.tensor_scalar(out=ot[:rows, 1::3], in0=m[:rows, :],
                                            scalar1=255.0, scalar2=None,
                                            op0=mybir.AluOpType.add)
                    nc.vector.tensor_scalar(out=m[:rows, :], in0=a_f[:rows, :],
                                            scalar1=0.0, scalar2=255.0,
                                            op0=mybir.AluOpType.is_equal,
                                            op1=mybir.AluOpType.mult)
                    nc.vector.tensor_scalar(out=ot[:rows, 2::3], in0=m[:rows, :],
                                            scalar1=255.0, scalar2=None,
                                            op0=mybir.AluOpType.add)
                    nc.sync.dma_start(
                        out=out_d.ap()[b, r0:r0 + rows, :, :]
                        .rearrange("p w c -> p (w c)"),
                        in_=ot[:rows, :],
                    )

            # emission order: decode both batches, batch-0 scatters, batch-0
            # dense (overlaps batch-1 scatters), batch-1 scatters, batch-1
            # dense. Tile's scheduler reorders within dependencies.
            decode_batch(0)
            decode_batch(1)
            last0 = scatter_batch(0)
            dense_batch(0, last0)
            last1 = scatter_batch(1)
            dense_batch(1, last1)

    nc.compile()
    return nc


def kernel(x: np.ndarray) -> np.ndarray:
    global _compiled
    from concourse.bass_utils import run_bass_kernel_spmd

    if _compiled is None:
        _compiled = _build()
    nc = _compiled

    x = np.ascontiguousarray(x, dtype=np.float32)
    in_maps = [{"x": x[c * BPC:(c + 1) * BPC]} for c in range(NCORES)]
    res = run_bass_kernel_spmd(nc, in_maps, list(range(NCORES)))
    out = np.concatenate([res.results[c]["out"] for c in range(NCORES)], axis=0)
    return out.astype(np.float32)
